# revision 21
# baseline (speedup 1.0000x reference)
"""Causal self-attention (B=4, T=2048, C=1024, 16 heads) on 8 TRN2 NeuronCores.

Sharding: data-parallel over batch (4) x tensor-parallel over heads (2 groups
of 8). Each core computes qkv + attention for its 8 heads and a partial
output projection (row-parallel); the host sums the two partials per batch.

Everything on-chip runs in a transposed layout so no tensor ever needs an
on-device transpose:
  QT/KT [ch, t]  <- W^T @ x^T      (x^T prepared on host)
  attT  [k, q]   = exp(K @ Q^T / 8) * causal_mask
  yT    [ch, q]  = V_aug^T @ attT  (V augmented with a ones column per head ->
                                    row 64 of each head's block = softmax denom)
  out   [q, c]   = yT^T @ Wp       (partial; host-reduced across head groups)

Softmax skips max-subtraction (scores/8 are O(1) here, exp is safe) which is
mathematically identical to the reference; masked-out blocks are never
computed (causal block skipping).
"""

import os
import sys

import numpy as np

for _p in ("/opt/trn_rl_repo", "/root/.axon_site/_ro/trn_rl_repo"):
    if os.path.isdir(_p) and _p not in sys.path:
        sys.path.append(_p)

import concourse.bass as bass  # noqa: E402
import concourse.mybir as mybir  # noqa: E402
import concourse.tile as tile  # noqa: E402
from concourse import bacc, bass_utils  # noqa: E402

f32 = mybir.dt.float32

B, T, C = 4, 2048, 1024
N_HEAD, D = 16, 64
NCORES = 8
HPC = 8  # heads per core
CH = HPC * D  # 512 channels per core
P = 128
NQ = 512  # q-strip width
NSTRIP = T // NQ  # 4
SCALE = 1.0 / 8.0  # 1/sqrt(D)

# Matmul operand dtype knob: f32 (exact, 4 cyc/row) or float32r (rounded
# TF32-like, 1 cyc/row at N>=256). Tensors feeding matmuls are declared in
# this dtype end-to-end (walrus requires fp32r matmul inputs to be produced
# as fp32r); the normalization broadcast stays exact f32.
MM_DT = mybir.dt.float32r


def build():
    nc = bacc.Bacc("TRN2", target_bir_lowering=False, debug=False)
    xt = nc.dram_tensor("xt", (C, T), MM_DT, kind="ExternalInput")
    wq = nc.dram_tensor("wq", (C, CH), MM_DT, kind="ExternalInput")
    wk = nc.dram_tensor("wk", (C, CH), MM_DT, kind="ExternalInput")
    wv = nc.dram_tensor("wv", (C, CH), MM_DT, kind="ExternalInput")
    wp = nc.dram_tensor("wp", (CH, C), MM_DT, kind="ExternalInput")
    mk = nc.dram_tensor("mk", (P, 128), f32, kind="ExternalInput")
    out = nc.dram_tensor("out", (T, C), f32, kind="ExternalOutput")
    Exp = mybir.ActivationFunctionType.Exp

    with tile.TileContext(nc) as tc:
        with (
            tc.tile_pool(name="sb", bufs=1) as sb,
            tc.tile_pool(name="ps", bufs=1, space="PSUM") as psp,
        ):
            mask = sb.tile([P, 128], f32, tag="mask", bufs=1, name="mask")
            nc.sync.dma_start(mask[:], mk[:])
            sel = sb.tile([33, P], f32, tag="sel", bufs=1, name="sel")
            nc.vector.memset(sel[:], 0.0)
            nc.vector.memset(sel[0:1, 0:64], 1.0)
            nc.vector.memset(sel[32:33, 64:128], 1.0)
            col1 = sb.tile([P, HPC], f32, tag="ones8", bufs=1, name="col1")
            nc.vector.memset(col1[:], 1.0)

            def load_w(dram, nm):
                ts_ = []
                for c in range(8):
                    t = sb.tile([P, CH], MM_DT, tag="w", bufs=32, name=f"{nm}{c}")
                    nc.sync.dma_start(t[:], dram[c * P : (c + 1) * P, :])
                    ts_.append(t)
                return ts_

            wq_sb = load_w(wq, "wq")
            wk_sb = load_w(wk, "wk")
            wv_sb = load_w(wv, "wv")
            wp_sb = []
            for c in range(4):
                for n in range(2):
                    t = sb.tile([P, NQ], MM_DT, tag="w", bufs=32, name=f"wp{c}{n}")
                    nc.sync.dma_start(t[:], wp[c * P : (c + 1) * P, n * NQ : (n + 1) * NQ])
                    wp_sb.append(t)

            kts = [sb.tile([P, T], MM_DT, tag="kt", bufs=4, name=f"kt{m}") for m in range(4)]
            vts = [None] * 16
            qts = {}

            # ---- Phases A+B interleaved per strip: compute QT/KT/V for strip
            # s, then attention for q-strip s (needs only K/V strips 0..s).
            # A(s+1)'s PE matmuls overlap B(s)'s ACT exps.
            GK = 3
            yts = {}

            def phase_a(s):
                xts = []
                for c in range(8):
                    t = sb.tile([P, NQ], MM_DT, tag="xy", bufs=16, name=f"x{s}_{c}")
                    nc.sync.dma_start(t[:], xt[c * P : (c + 1) * P, s * NQ : (s + 1) * NQ])
                    xts.append(t)
                qts[s] = []
                for m in range(4):
                    ps = psp.tile([P, NQ], f32, tag="mm", bufs=3, name="psa")
                    for c in range(8):
                        nc.tensor.matmul(
                            ps[:],
                            wq_sb[c][:, m * P : (m + 1) * P],
                            xts[c][:],
                            start=(c == 0),
                            stop=(c == 7),
                        )
                    qt_t = sb.tile([P, NQ], MM_DT, tag="qt", bufs=8, name=f"q{s}_{m}")
                    nc.vector.tensor_copy(qt_t[:], ps[:])
                    qts[s].append(qt_t)
                for m in range(4):
                    ps = psp.tile([P, NQ], f32, tag="mm", bufs=3, name="psk")
                    for c in range(8):
                        nc.tensor.matmul(
                            ps[:],
                            wk_sb[c][:, m * P : (m + 1) * P],
                            xts[c][:],
                            start=(c == 0),
                            stop=(c == 7),
                        )
                    nc.vector.tensor_copy(kts[m][:, s * NQ : (s + 1) * NQ], ps[:])
                for mt in range(4):
                    g = s * 4 + mt
                    ps = psp.tile([P, NQ], f32, tag="mm", bufs=3, name="psv")
                    for c in range(8):
                        nc.tensor.matmul(
                            ps[:],
                            xts[c][:, mt * P : (mt + 1) * P],
                            wv_sb[c][:],
                            start=(c == 0),
                            stop=(c == 7),
                        )
                    vt = sb.tile([P, HPC * 65], MM_DT, tag="v", bufs=16, name=f"v{g}")
                    v3 = vt.rearrange("p (h e) -> p h e", e=65)
                    nc.vector.tensor_copy(v3[:, :, 0:64], ps.rearrange("p (h e) -> p h e", e=64))
                    nc.vector.tensor_copy(
                        v3[:, :, 64:65], col1[:].rearrange("p (h e) -> p h e", e=1)
                    )
                    vts[g] = vt

            def phase_b(s):
                for c in range(4):  # head pairs
                    av = [
                        psp.tile([65, NQ], f32, tag="sm", bufs=2, name=f"av{s}{c}{u}")
                        for u in range(2)
                    ]
                    nkt = 4 * (s + 1)
                    for g0 in range(0, nkt, GK):
                        grp = range(g0, min(g0 + GK, nkt))
                        atts = {}
                        for kt in grp:
                            j = kt - 4 * s  # >=0: diagonal-straddling tile
                            off = 128 * max(j, 0)
                            # both heads' scores side by side in one 2-bank tile
                            qkp = psp.tile([P, 2 * NQ], f32, tag="mm", bufs=3, name="qkp")
                            for sub in range(2):
                                nc.tensor.matmul(
                                    qkp[:, sub * NQ + off : (sub + 1) * NQ],
                                    kts[c][sub * 64 : (sub + 1) * 64, kt * P : (kt + 1) * P],
                                    qts[s][c][sub * 64 : (sub + 1) * 64, off:NQ],
                                    start=True,
                                    stop=True,
                                )
                            att = sb.tile([P, 2 * NQ], MM_DT, tag="att", bufs=GK + 1, name="att")
                            # one exp over both heads' valid column ranges
                            nc.scalar.activation(
                                att.rearrange("p (u q) -> p u q", u=2)[:, :, off:NQ],
                                qkp.rearrange("p (u q) -> p u q", u=2)[:, :, off:NQ],
                                Exp,
                                scale=SCALE,
                            )
                            if j >= 0:
                                for sub in range(2):
                                    nc.vector.tensor_mul(
                                        att[:, sub * NQ + off : sub * NQ + off + 128],
                                        att[:, sub * NQ + off : sub * NQ + off + 128],
                                        mask[:],
                                    )
                            atts[kt] = (att, off)
                        for kt in grp:
                            att, off = atts[kt]
                            for sub in range(2):
                                h = 2 * c + sub
                                nc.tensor.matmul(
                                    av[sub][:, off:NQ],
                                    vts[kt][:, h * 65 : (h + 1) * 65],
                                    att[:, sub * NQ + off : (sub + 1) * NQ],
                                    start=(kt == 0),
                                    stop=(kt == nkt - 1),
                                )
                    # normalize: rc2 = 1/denoms (both heads), broadcast via one
                    # selector matmul to [128, NQ], then scale yT
                    rc2 = sb.tile([33, NQ], f32, tag="rc2", bufs=2, name="rc2")
                    scr = sb.tile([33, NQ], f32, tag="scr", bufs=1, name="scr")
                    den = sb.tile([33, NQ], f32, tag="den", bufs=1, name="den")
                    nc.vector.memset(den[:], 1.0)
                    for sub in range(2):
                        nc.vector.tensor_copy(den[32 * sub : 32 * sub + 1, :], av[sub][64:65, :])
                    nc.vector.reciprocal_approx_accurate(out=rc2[:], in_=den[:], scratch=scr[:])
                    bc_ps = psp.tile([P, NQ], f32, tag="mm", bufs=3, name="bcp")
                    nc.tensor.matmul(bc_ps[:], sel[:], rc2[:], start=True, stop=True)
                    bc = sb.tile([P, NQ], f32, tag="bc", bufs=1, name="bc")
                    nc.vector.tensor_copy(bc[:], bc_ps[:])
                    yts[(c, s)] = sb.tile([P, NQ], MM_DT, tag="xy", bufs=16, name=f"y{c}{s}")
                    for sub in range(2):
                        nc.vector.tensor_mul(
                            yts[(c, s)][sub * 64 : (sub + 1) * 64, :],
                            av[sub][0:64, :],
                            bc[sub * 64 : (sub + 1) * 64, :],
                        )

            def phase_c(s):
                for o in range(4):
                    m = 4 * s + o
                    for n in range(2):
                        ps = psp.tile([P, NQ], f32, tag="mm", bufs=3, name="psc")
                        for c in range(4):
                            nc.tensor.matmul(
                                ps[:],
                                yts[(c, s)][:, o * P : (o + 1) * P],
                                wp_sb[c * 2 + n][:],
                                start=(c == 0),
                                stop=(c == 3),
                            )
                        ot = sb.tile([P, NQ], f32, tag="att", bufs=GK + 1, name="ot")
                        nc.scalar.copy(ot[:], ps[:])
                        nc.sync.dma_start(out[m * P : (m + 1) * P, n * NQ : (n + 1) * NQ], ot[:])

            for s in range(NSTRIP):
                phase_a(s)
                phase_b(s)
                phase_c(s)

    nc.compile()
    return nc


_NC = None


def _get_nc():
    global _NC
    if _NC is None:
        _NC = build()
    return _NC


def host_mask():
    # diagonal-block causal mask: keep k <= q within a 128x128 block
    m = np.zeros((P, P), np.float32)
    for kk in range(P):
        m[kk, kk:] = 1.0
    return m


def make_in_maps(x, w_qkv, w_proj):
    x = np.asarray(x, np.float32)
    w_qkv = np.asarray(w_qkv, np.float32)
    w_proj = np.asarray(w_proj, np.float32)
    mkm = host_mask()
    in_maps = []
    for core in range(NCORES):
        b, hg = core // 2, core % 2
        lo, hi = hg * CH, (hg + 1) * CH
        in_maps.append(
            {
                "xt": np.ascontiguousarray(x[b].T),
                "wq": np.ascontiguousarray(w_qkv[:, lo:hi]),
                "wk": np.ascontiguousarray(w_qkv[:, C + lo : C + hi]),
                "wv": np.ascontiguousarray(w_qkv[:, 2 * C + lo : 2 * C + hi]),
                "wp": np.ascontiguousarray(w_proj[lo:hi, :]),
                "mk": mkm,
            }
        )
    return in_maps


def kernel(x, w_qkv, w_proj):
    in_maps = make_in_maps(x, w_qkv, w_proj)
    res = bass_utils.run_bass_kernel_spmd(_get_nc(), in_maps, core_ids=list(range(NCORES)))
    out = np.empty((B, T, C), np.float32)
    for b in range(B):
        out[b] = res.results[2 * b]["out"] + res.results[2 * b + 1]["out"]
    return out


# revision 22
# speedup vs baseline: 1.0248x; 1.0248x over previous
"""Causal self-attention (B=4, T=2048, C=1024, 16 heads) on 8 TRN2 NeuronCores.

Sharding: data-parallel over batch (4) x tensor-parallel over heads (2 groups
of 8). Each core computes qkv + attention for its 8 heads and a partial
output projection (row-parallel); the host sums the two partials per batch.

Everything on-chip runs in a transposed layout so no tensor ever needs an
on-device transpose:
  QT/KT [ch, t]  <- W^T @ x^T      (x^T prepared on host)
  attT  [k, q]   = exp(K @ Q^T / 8) * causal_mask
  yT    [ch, q]  = V_aug^T @ attT  (V augmented with a ones column per head ->
                                    row 64 of each head's block = softmax denom)
  out   [q, c]   = yT^T @ Wp       (partial; host-reduced across head groups)

Softmax skips max-subtraction (scores/8 are O(1) here, exp is safe) which is
mathematically identical to the reference; masked-out blocks are never
computed (causal block skipping).
"""

import os
import sys

import numpy as np

for _p in ("/opt/trn_rl_repo", "/root/.axon_site/_ro/trn_rl_repo"):
    if os.path.isdir(_p) and _p not in sys.path:
        sys.path.append(_p)

import concourse.bass as bass  # noqa: E402
import concourse.mybir as mybir  # noqa: E402
import concourse.tile as tile  # noqa: E402
from concourse import bacc, bass_utils  # noqa: E402

f32 = mybir.dt.float32

B, T, C = 4, 2048, 1024
N_HEAD, D = 16, 64
NCORES = 8
HPC = 8  # heads per core
CH = HPC * D  # 512 channels per core
P = 128
NQ = 512  # q-strip width
NSTRIP = T // NQ  # 4
SCALE = 1.0 / 8.0  # 1/sqrt(D)

# Matmul operand dtype knob: f32 (exact, 4 cyc/row) or float32r (rounded
# TF32-like, 1 cyc/row at N>=256). Tensors feeding matmuls are declared in
# this dtype end-to-end (walrus requires fp32r matmul inputs to be produced
# as fp32r); the normalization broadcast stays exact f32.
MM_DT = mybir.dt.float32r


def build():
    nc = bacc.Bacc("TRN2", target_bir_lowering=False, debug=False)
    xt = nc.dram_tensor("xt", (C, T), MM_DT, kind="ExternalInput")
    wq = nc.dram_tensor("wq", (C, CH), MM_DT, kind="ExternalInput")
    wk = nc.dram_tensor("wk", (C, CH), MM_DT, kind="ExternalInput")
    wv = nc.dram_tensor("wv", (C, CH), MM_DT, kind="ExternalInput")
    wp = nc.dram_tensor("wp", (CH, C), MM_DT, kind="ExternalInput")
    mk = nc.dram_tensor("mk", (P, 128), f32, kind="ExternalInput")
    out = nc.dram_tensor("out", (T, C), f32, kind="ExternalOutput")
    Exp = mybir.ActivationFunctionType.Exp

    with tile.TileContext(nc) as tc:
        with (
            tc.tile_pool(name="sb", bufs=1) as sb,
            tc.tile_pool(name="ps", bufs=1, space="PSUM") as psp,
        ):
            mask = sb.tile([P, 128], f32, tag="mask", bufs=1, name="mask")
            nc.sync.dma_start(mask[:], mk[:])
            sel = sb.tile([33, P], f32, tag="sel", bufs=1, name="sel")
            nc.vector.memset(sel[:], 0.0)
            nc.vector.memset(sel[0:1, 0:64], 1.0)
            nc.vector.memset(sel[32:33, 64:128], 1.0)
            col1 = sb.tile([P, HPC], f32, tag="ones8", bufs=1, name="col1")
            nc.vector.memset(col1[:], 1.0)

            def load_w(dram, nm):
                ts_ = []
                for c in range(8):
                    t = sb.tile([P, CH], MM_DT, tag="w", bufs=32, name=f"{nm}{c}")
                    nc.sync.dma_start(t[:], dram[c * P : (c + 1) * P, :])
                    ts_.append(t)
                return ts_

            wq_sb = load_w(wq, "wq")
            wk_sb = load_w(wk, "wk")
            wv_sb = load_w(wv, "wv")
            wp_sb = []
            for c in range(4):
                for n in range(2):
                    t = sb.tile([P, NQ], MM_DT, tag="w", bufs=32, name=f"wp{c}{n}")
                    nc.sync.dma_start(t[:], wp[c * P : (c + 1) * P, n * NQ : (n + 1) * NQ])
                    wp_sb.append(t)

            kts = [sb.tile([P, T], MM_DT, tag="kt", bufs=4, name=f"kt{m}") for m in range(4)]
            vts = [None] * 16
            qts = {}

            # ---- Phases A+B interleaved per strip: compute QT/KT/V for strip
            # s, then attention for q-strip s (needs only K/V strips 0..s).
            # A(s+1)'s PE matmuls overlap B(s)'s ACT exps.
            GK = 3
            yts = {}

            def phase_a(s):
                xts = []
                for c in range(8):
                    t = sb.tile([P, NQ], MM_DT, tag="xy", bufs=16, name=f"x{s}_{c}")
                    nc.sync.dma_start(t[:], xt[c * P : (c + 1) * P, s * NQ : (s + 1) * NQ])
                    xts.append(t)
                qts[s] = []
                for m in range(4):
                    ps = psp.tile([P, NQ], f32, tag="mm", bufs=2, name="psa")
                    for c in range(8):
                        nc.tensor.matmul(
                            ps[:],
                            wq_sb[c][:, m * P : (m + 1) * P],
                            xts[c][:],
                            start=(c == 0),
                            stop=(c == 7),
                        )
                    qt_t = sb.tile([P, NQ], MM_DT, tag="qt", bufs=8, name=f"q{s}_{m}")
                    nc.vector.tensor_copy(qt_t[:], ps[:])
                    qts[s].append(qt_t)
                for m in range(4):
                    ps = psp.tile([P, NQ], f32, tag="mm", bufs=2, name="psk")
                    for c in range(8):
                        nc.tensor.matmul(
                            ps[:],
                            wk_sb[c][:, m * P : (m + 1) * P],
                            xts[c][:],
                            start=(c == 0),
                            stop=(c == 7),
                        )
                    nc.vector.tensor_copy(kts[m][:, s * NQ : (s + 1) * NQ], ps[:])
                for mt in range(4):
                    g = s * 4 + mt
                    ps = psp.tile([P, NQ], f32, tag="mm", bufs=2, name="psv")
                    for c in range(8):
                        nc.tensor.matmul(
                            ps[:],
                            xts[c][:, mt * P : (mt + 1) * P],
                            wv_sb[c][:],
                            start=(c == 0),
                            stop=(c == 7),
                        )
                    vt = sb.tile([P, HPC * 65], MM_DT, tag="v", bufs=16, name=f"v{g}")
                    v3 = vt.rearrange("p (h e) -> p h e", e=65)
                    nc.vector.tensor_copy(v3[:, :, 0:64], ps.rearrange("p (h e) -> p h e", e=64))
                    nc.vector.tensor_copy(
                        v3[:, :, 64:65], col1[:].rearrange("p (h e) -> p h e", e=1)
                    )
                    vts[g] = vt

            def phase_b(s):
                for c in range(4):  # head pairs
                    av = [
                        psp.tile([65, NQ], f32, tag="sm", bufs=2, name=f"av{s}{c}{u}")
                        for u in range(2)
                    ]
                    nkt = 4 * (s + 1)
                    for g0 in range(0, nkt, GK):
                        grp = range(g0, min(g0 + GK, nkt))
                        atts = {}
                        for kt in grp:
                            j = kt - 4 * s  # >=0: diagonal-straddling tile
                            off = 128 * max(j, 0)
                            # both heads' scores side by side in one 2-bank tile
                            qkp = psp.tile([P, 2 * NQ], f32, tag="qk", bufs=2, name="qkp")
                            for sub in range(2):
                                nc.tensor.matmul(
                                    qkp[:, sub * NQ + off : (sub + 1) * NQ],
                                    kts[c][sub * 64 : (sub + 1) * 64, kt * P : (kt + 1) * P],
                                    qts[s][c][sub * 64 : (sub + 1) * 64, off:NQ],
                                    start=True,
                                    stop=True,
                                )
                            att = sb.tile([P, 2 * NQ], MM_DT, tag="att", bufs=GK + 1, name="att")
                            # one exp over both heads' valid column ranges
                            nc.scalar.activation(
                                att.rearrange("p (u q) -> p u q", u=2)[:, :, off:NQ],
                                qkp.rearrange("p (u q) -> p u q", u=2)[:, :, off:NQ],
                                Exp,
                                scale=SCALE,
                            )
                            if j >= 0:
                                for sub in range(2):
                                    nc.vector.tensor_mul(
                                        att[:, sub * NQ + off : sub * NQ + off + 128],
                                        att[:, sub * NQ + off : sub * NQ + off + 128],
                                        mask[:],
                                    )
                            atts[kt] = (att, off)
                        for kt in grp:
                            att, off = atts[kt]
                            for sub in range(2):
                                h = 2 * c + sub
                                nc.tensor.matmul(
                                    av[sub][:, off:NQ],
                                    vts[kt][:, h * 65 : (h + 1) * 65],
                                    att[:, sub * NQ + off : (sub + 1) * NQ],
                                    start=(kt == 0),
                                    stop=(kt == nkt - 1),
                                )
                    # normalize: rc2 = 1/denoms (both heads), broadcast via one
                    # selector matmul to [128, NQ], then scale yT
                    rc2 = sb.tile([33, NQ], f32, tag="rc2", bufs=2, name="rc2")
                    scr = sb.tile([33, NQ], f32, tag="scr", bufs=1, name="scr")
                    den = sb.tile([33, NQ], f32, tag="den", bufs=1, name="den")
                    nc.vector.memset(den[:], 1.0)
                    for sub in range(2):
                        nc.vector.tensor_copy(den[32 * sub : 32 * sub + 1, :], av[sub][64:65, :])
                    nc.vector.reciprocal_approx_accurate(out=rc2[:], in_=den[:], scratch=scr[:])
                    bc_ps = psp.tile([P, NQ], f32, tag="mm", bufs=2, name="bcp")
                    nc.tensor.matmul(bc_ps[:], sel[:], rc2[:], start=True, stop=True)
                    bc = sb.tile([P, NQ], f32, tag="bc", bufs=1, name="bc")
                    nc.vector.tensor_copy(bc[:], bc_ps[:])
                    yts[(c, s)] = sb.tile([P, NQ], MM_DT, tag="xy", bufs=16, name=f"y{c}{s}")
                    for sub in range(2):
                        nc.vector.tensor_mul(
                            yts[(c, s)][sub * 64 : (sub + 1) * 64, :],
                            av[sub][0:64, :],
                            bc[sub * 64 : (sub + 1) * 64, :],
                        )

            def phase_c(s):
                for o in range(4):
                    m = 4 * s + o
                    for n in range(2):
                        ps = psp.tile([P, NQ], f32, tag="mm", bufs=2, name="psc")
                        for c in range(4):
                            nc.tensor.matmul(
                                ps[:],
                                yts[(c, s)][:, o * P : (o + 1) * P],
                                wp_sb[c * 2 + n][:],
                                start=(c == 0),
                                stop=(c == 3),
                            )
                        ot = sb.tile([P, NQ], f32, tag="att", bufs=GK + 1, name="ot")
                        nc.vector.tensor_copy(ot[:], ps[:])
                        nc.sync.dma_start(out[m * P : (m + 1) * P, n * NQ : (n + 1) * NQ], ot[:])

            for s in range(NSTRIP):
                phase_a(s)
                phase_b(s)
                phase_c(s)

    nc.compile()
    return nc


_NC = None


def _get_nc():
    global _NC
    if _NC is None:
        _NC = build()
    return _NC


def host_mask():
    # diagonal-block causal mask: keep k <= q within a 128x128 block
    m = np.zeros((P, P), np.float32)
    for kk in range(P):
        m[kk, kk:] = 1.0
    return m


def make_in_maps(x, w_qkv, w_proj):
    x = np.asarray(x, np.float32)
    w_qkv = np.asarray(w_qkv, np.float32)
    w_proj = np.asarray(w_proj, np.float32)
    mkm = host_mask()
    in_maps = []
    for core in range(NCORES):
        b, hg = core // 2, core % 2
        lo, hi = hg * CH, (hg + 1) * CH
        in_maps.append(
            {
                "xt": np.ascontiguousarray(x[b].T),
                "wq": np.ascontiguousarray(w_qkv[:, lo:hi]),
                "wk": np.ascontiguousarray(w_qkv[:, C + lo : C + hi]),
                "wv": np.ascontiguousarray(w_qkv[:, 2 * C + lo : 2 * C + hi]),
                "wp": np.ascontiguousarray(w_proj[lo:hi, :]),
                "mk": mkm,
            }
        )
    return in_maps


def kernel(x, w_qkv, w_proj):
    in_maps = make_in_maps(x, w_qkv, w_proj)
    res = bass_utils.run_bass_kernel_spmd(_get_nc(), in_maps, core_ids=list(range(NCORES)))
    out = np.empty((B, T, C), np.float32)
    for b in range(B):
        out[b] = res.results[2 * b]["out"] + res.results[2 * b + 1]["out"]
    return out


# revision 23
# speedup vs baseline: 1.1330x; 1.1056x over previous
"""Causal self-attention (B=4, T=2048, C=1024, 16 heads) on 8 TRN2 NeuronCores.

Sharding: data-parallel over batch (4) x tensor-parallel over heads (2 groups
of 8). Each core computes qkv + attention for its 8 heads and a partial
output projection (row-parallel); the host sums the two partials per batch.

Everything on-chip runs in a transposed layout so no tensor ever needs an
on-device transpose:
  QT/KT [ch, t]  <- W^T @ x^T      (x^T prepared on host)
  attT  [k, q]   = exp(K @ Q^T / 8) * causal_mask
  yT    [ch, q]  = V_aug^T @ attT  (V augmented with a ones column per head ->
                                    row 64 of each head's block = softmax denom)
  out   [q, c]   = yT^T @ Wp       (partial; host-reduced across head groups)

Softmax skips max-subtraction (scores/8 are O(1) here, exp is safe), which is
mathematically identical to the reference; fully-masked blocks are never
computed, straddle blocks only compute the causally valid column range, and
only the diagonal 128-wide sub-block needs a mask multiply.

Matmuls run in float32r (TF32-like, 1 cyc/row at N>=256; ~1.5e-4 rel err);
the softmax normalization (reciprocal + broadcast) stays exact f32.
"""

import os
import sys

import numpy as np

for _p in ("/opt/trn_rl_repo", "/root/.axon_site/_ro/trn_rl_repo"):
    if os.path.isdir(_p) and _p not in sys.path:
        sys.path.append(_p)

import concourse.bass as bass  # noqa: E402,F401
import concourse.mybir as mybir  # noqa: E402
import concourse.tile as tile  # noqa: E402
from concourse import bacc, bass_utils  # noqa: E402

f32 = mybir.dt.float32

B, T, C = 4, 2048, 1024
N_HEAD, D = 16, 64
NCORES = 8
HPC = 8  # heads per core
CH = HPC * D  # 512 channels per core
P = 128
NQ = 512  # q-strip width
NSTRIP = T // NQ  # 4
SCALE = 1.0 / 8.0  # 1/sqrt(D)
GK = 3  # k-tiles per QK/AV group

MM_DT = mybir.dt.float32r


def build():
    nc = bacc.Bacc("TRN2", target_bir_lowering=False, debug=False)
    xt = nc.dram_tensor("xt", (C, T), MM_DT, kind="ExternalInput")
    wq = nc.dram_tensor("wq", (C, CH), MM_DT, kind="ExternalInput")
    wk = nc.dram_tensor("wk", (C, CH), MM_DT, kind="ExternalInput")
    wv = nc.dram_tensor("wv", (C, CH), MM_DT, kind="ExternalInput")
    wp = nc.dram_tensor("wp", (CH, C), MM_DT, kind="ExternalInput")
    mk = nc.dram_tensor("mk", (P, 128), f32, kind="ExternalInput")
    out = nc.dram_tensor("out", (T, C), f32, kind="ExternalOutput")
    Exp = mybir.ActivationFunctionType.Exp

    with tile.TileContext(nc) as tc:
        with (
            tc.tile_pool(name="sb", bufs=1) as sb,
            tc.tile_pool(name="ps", bufs=1, space="PSUM") as psp,
        ):
            mask = sb.tile([P, 128], f32, tag="mask", bufs=1, name="mask")
            nc.sync.dma_start(mask[:], mk[:])
            sel = sb.tile([33, P], f32, tag="sel", bufs=1, name="sel")
            nc.vector.memset(sel[:], 0.0)
            nc.vector.memset(sel[0:1, 0:64], 1.0)
            nc.vector.memset(sel[32:33, 64:128], 1.0)
            col1 = sb.tile([P, HPC], f32, tag="ones8", bufs=1, name="col1")
            nc.vector.memset(col1[:], 1.0)

            def load_w(dram, nm):
                ts_ = []
                for c in range(8):
                    t = sb.tile([P, CH], MM_DT, tag="w", bufs=24, name=f"{nm}{c}")
                    nc.sync.dma_start(t[:], dram[c * P : (c + 1) * P, :])
                    ts_.append(t)
                return ts_

            wq_sb = load_w(wq, "wq")
            wk_sb = load_w(wk, "wk")
            wv_sb = load_w(wv, "wv")

            qts = [sb.tile([P, T], MM_DT, tag="qt", bufs=4, name=f"qt{m}") for m in range(4)]
            kts = [sb.tile([P, T], MM_DT, tag="kt", bufs=4, name=f"kt{m}") for m in range(4)]
            vts = [None] * 16

            # ---- Phase A: QT, KT (transposed) and V (natural, ones-augmented)
            for s in range(NSTRIP):
                xts = []
                for c in range(8):
                    t = sb.tile([P, NQ], MM_DT, tag="xy", bufs=16, name=f"x{s}_{c}")
                    nc.sync.dma_start(t[:], xt[c * P : (c + 1) * P, s * NQ : (s + 1) * NQ])
                    xts.append(t)
                for w_sb, dst in ((wq_sb, qts), (wk_sb, kts)):
                    for m in range(4):
                        ps = psp.tile([P, NQ], f32, tag="mm", bufs=2, name="psa")
                        for c in range(8):
                            nc.tensor.matmul(
                                ps[:],
                                w_sb[c][:, m * P : (m + 1) * P],
                                xts[c][:],
                                start=(c == 0),
                                stop=(c == 7),
                            )
                        nc.vector.tensor_copy(dst[m][:, s * NQ : (s + 1) * NQ], ps[:])
                for mt in range(4):
                    g = s * 4 + mt
                    ps = psp.tile([P, NQ], f32, tag="mm", bufs=2, name="psv")
                    for c in range(8):
                        nc.tensor.matmul(
                            ps[:],
                            xts[c][:, mt * P : (mt + 1) * P],
                            wv_sb[c][:],
                            start=(c == 0),
                            stop=(c == 7),
                        )
                    vt = sb.tile([P, HPC * 65], MM_DT, tag="v", bufs=16, name=f"v{g}")
                    v3 = vt.rearrange("p (h e) -> p h e", e=65)
                    nc.vector.tensor_copy(v3[:, :, 0:64], ps.rearrange("p (h e) -> p h e", e=64))
                    nc.vector.tensor_copy(
                        v3[:, :, 64:65], col1[:].rearrange("p (h e) -> p h e", e=1)
                    )
                    vts[g] = vt

            # ---- Phase B: flash attention in transposed layout.
            # QK pairs (sub 0/1 = PE row groups 0-1/2-3) issue adjacently so
            # they run concurrently; AV matmuls for a group of GK k-tiles run
            # as one uninterrupted chain (avoids per-MM weight-switch stalls).
            yts = {}
            for s in range(NSTRIP):
                for c in range(4):  # head pairs
                    av = [
                        psp.tile([65, NQ], f32, tag="sm", bufs=2, name=f"av{s}{c}{u}")
                        for u in range(2)
                    ]
                    nkt = 4 * (s + 1)
                    for g0 in range(0, nkt, GK):
                        grp = range(g0, min(g0 + GK, nkt))
                        atts = {}
                        for kt in grp:
                            j = kt - 4 * s  # >=0: diagonal-straddling tile
                            off = 128 * max(j, 0)
                            # both heads' scores side by side in one 2-bank tile
                            qkp = psp.tile([P, 2 * NQ], f32, tag="qk", bufs=2, name="qkp")
                            for sub in range(2):
                                nc.tensor.matmul(
                                    qkp[:, sub * NQ + off : (sub + 1) * NQ],
                                    kts[c][sub * 64 : (sub + 1) * 64, kt * P : (kt + 1) * P],
                                    qts[c][sub * 64 : (sub + 1) * 64, s * NQ + off : (s + 1) * NQ],
                                    start=True,
                                    stop=True,
                                )
                            att = sb.tile([P, 2 * NQ], MM_DT, tag="att", bufs=GK + 1, name="att")
                            # one exp over both heads' valid column ranges
                            nc.scalar.activation(
                                att.rearrange("p (u q) -> p u q", u=2)[:, :, off:NQ],
                                qkp.rearrange("p (u q) -> p u q", u=2)[:, :, off:NQ],
                                Exp,
                                scale=SCALE,
                            )
                            if j >= 0:
                                for sub in range(2):
                                    nc.vector.tensor_mul(
                                        att[:, sub * NQ + off : sub * NQ + off + 128],
                                        att[:, sub * NQ + off : sub * NQ + off + 128],
                                        mask[:],
                                    )
                            atts[kt] = (att, off)
                        for kt in grp:
                            att, off = atts[kt]
                            for sub in range(2):
                                h = 2 * c + sub
                                nc.tensor.matmul(
                                    av[sub][:, off:NQ],
                                    vts[kt][:, h * 65 : (h + 1) * 65],
                                    att[:, sub * NQ + off : (sub + 1) * NQ],
                                    start=(kt == 0),
                                    stop=(kt == nkt - 1),
                                )
                    # normalize: rc2 = 1/denoms (both heads), broadcast via one
                    # selector matmul to [128, NQ], then scale yT
                    rc2 = sb.tile([33, NQ], f32, tag="rc2", bufs=2, name="rc2")
                    scr = sb.tile([33, NQ], f32, tag="scr", bufs=1, name="scr")
                    den = sb.tile([33, NQ], f32, tag="den", bufs=1, name="den")
                    nc.vector.memset(den[:], 1.0)
                    for sub in range(2):
                        nc.vector.tensor_copy(den[32 * sub : 32 * sub + 1, :], av[sub][64:65, :])
                    nc.vector.reciprocal_approx_accurate(out=rc2[:], in_=den[:], scratch=scr[:])
                    bc_ps = psp.tile([P, NQ], f32, tag="mm", bufs=2, name="bcp")
                    nc.tensor.matmul(bc_ps[:], sel[:], rc2[:], start=True, stop=True)
                    bc = sb.tile([P, NQ], f32, tag="bc", bufs=1, name="bc")
                    nc.vector.tensor_copy(bc[:], bc_ps[:])
                    yts[(c, s)] = sb.tile([P, NQ], MM_DT, tag="xy", bufs=16, name=f"y{c}{s}")
                    for sub in range(2):
                        nc.vector.tensor_mul(
                            yts[(c, s)][sub * 64 : (sub + 1) * 64, :],
                            av[sub][0:64, :],
                            bc[sub * 64 : (sub + 1) * 64, :],
                        )

            # ---- Phase C: partial projection (host reduces across head groups)
            wp_sb = []
            for c in range(4):
                for n in range(2):
                    t = sb.tile([P, NQ], MM_DT, tag="w", bufs=24, name=f"wp{c}{n}")
                    nc.sync.dma_start(t[:], wp[c * P : (c + 1) * P, n * NQ : (n + 1) * NQ])
                    wp_sb.append(t)
            for m in range(16):
                s, o = m // 4, m % 4
                for n in range(2):
                    ps = psp.tile([P, NQ], f32, tag="mm", bufs=2, name="psc")
                    for c in range(4):
                        nc.tensor.matmul(
                            ps[:],
                            yts[(c, s)][:, o * P : (o + 1) * P],
                            wp_sb[c * 2 + n][:],
                            start=(c == 0),
                            stop=(c == 3),
                        )
                    ot = sb.tile([P, NQ], f32, tag="att", bufs=GK + 1, name="ot")
                    nc.vector.tensor_copy(ot[:], ps[:])
                    nc.sync.dma_start(out[m * P : (m + 1) * P, n * NQ : (n + 1) * NQ], ot[:])

    nc.compile()
    return nc


_NC = None


def _get_nc():
    global _NC
    if _NC is None:
        _NC = build()
    return _NC


def host_mask():
    # diagonal-block causal mask: keep k <= q within a 128x128 block
    m = np.zeros((P, P), np.float32)
    for kk in range(P):
        m[kk, kk:] = 1.0
    return m


def make_in_maps(x, w_qkv, w_proj):
    x = np.asarray(x, np.float32)
    w_qkv = np.asarray(w_qkv, np.float32)
    w_proj = np.asarray(w_proj, np.float32)
    mkm = host_mask()
    in_maps = []
    for core in range(NCORES):
        b, hg = core // 2, core % 2
        lo, hi = hg * CH, (hg + 1) * CH
        in_maps.append(
            {
                "xt": np.ascontiguousarray(x[b].T),
                "wq": np.ascontiguousarray(w_qkv[:, lo:hi]),
                "wk": np.ascontiguousarray(w_qkv[:, C + lo : C + hi]),
                "wv": np.ascontiguousarray(w_qkv[:, 2 * C + lo : 2 * C + hi]),
                "wp": np.ascontiguousarray(w_proj[lo:hi, :]),
                "mk": mkm,
            }
        )
    return in_maps


def kernel(x, w_qkv, w_proj):
    in_maps = make_in_maps(x, w_qkv, w_proj)
    res = bass_utils.run_bass_kernel_spmd(_get_nc(), in_maps, core_ids=list(range(NCORES)))
    out = np.empty((B, T, C), np.float32)
    for b in range(B):
        out[b] = res.results[2 * b]["out"] + res.results[2 * b + 1]["out"]
    return out
